# revision 17
# baseline (speedup 1.0000x reference)
"""Trainium2 Bass kernel for nn_ContinuousGraphDiffusion.

kernel(**inputs) takes the FULL inputs from reference.setup_inputs() as
numpy arrays and returns the FULL output (scalar loss).  Sharding:

  Phase A (data-parallel over batch): each of the 8 cores runs the GNN
  for its 4 samples, in "F-major" (feature-in-partition) layout so the
  adjacency is only needed in transposed [j, i] form (host supplies it
  pre-transposed; pure layout prep).  Row-normalization is applied after
  the message matmul via a PE outer-product broadcast, msg_b is folded in
  as a rank-1 PE update, LayerNorm stats are computed with ones-vector
  matmuls and applied with outer-product scale/shift maps.

  Phase B (tensor-parallel over the 130816 triu edges): triangle rows
  interleaved round-robin (core c owns rows i == c mod 8).  The host
  scatters op_W2 columns into a zero-padded [256, 64, 512] per-core block
  (bf16) whose (k, j) column is the weight of edge (row_k, j); targets
  are full noise rows (host-sliced), masked on device.  One small
  AllGather shares the pooled graph features; each core returns its
  partial sum-of-squares / (B*NE); the host sums the 8 partials.
"""

import sys
from contextlib import ExitStack

import numpy as np

sys.path.insert(0, "/opt/trn_rl_repo")

import concourse.bass as bass  # noqa: E402
import concourse.tile as tile  # noqa: E402
from concourse import bacc, mybir  # noqa: E402
from concourse.bass_utils import run_bass_kernel_spmd  # noqa: E402

try:
    from ml_dtypes import bfloat16 as np_bf16
except Exception:  # pragma: no cover
    np_bf16 = None

FP = mybir.dt.float32
BF = mybir.dt.bfloat16
I32 = mybir.dt.int32

N = 512
H = 256
L = 4
T = 100
B = 32
NC = 8
BS = B // NC           # samples per core
NE = N * (N - 1) // 2  # 130816
NROW = N // NC         # 64 triangle rows per core
NT = NROW // 4         # 16 packed [128, 512] target tiles per core
AF = mybir.ActivationFunctionType
ALU = mybir.AluOpType

# cosine noise schedule (compile-time constants; same formula as the model)
_s = 0.008
_steps = np.arange(T + 1, dtype=np.float64)
_ab = np.cos((_steps / T + _s) / (1 + _s) * np.pi / 2) ** 2
_ab = _ab / _ab[0]
_betas = np.clip(1.0 - _ab[1:] / _ab[:-1], None, 0.999).astype(np.float32)
_alphas = 1.0 - _betas
_alpha_bar = np.cumprod(_alphas).astype(np.float32)
SQRT_AB = np.sqrt(_alpha_bar).astype(np.float32)
SQRT_OMAB = np.sqrt(1.0 - _alpha_bar).astype(np.float32)

DEBUG = False
DBG_SPECS = [("dbg_sasb", [2, BS]), ("dbg_rinv", [1, N]),
             ("dbg_h0", [128, 2, N]), ("dbg_hf", [128, 2, N]),
             ("dbg_gof", [128, 2, BS]), ("dbg_d0", [128, N]),
             ("dbg_sse", [128, NT])]


def _edge_index():
    """EIDX[i, j] = linear triu edge index of (i, j) for j > i, else NE."""
    off = np.zeros(N, dtype=np.int64)
    lens = (N - 1) - np.arange(N)
    off[1:] = np.cumsum(lens[:-1])
    j = np.arange(N)[None, :]
    i = np.arange(N)[:, None]
    e = off[:, None] + (j - i - 1)
    return np.where(j > i, e, NE)


EIDX = _edge_index()


# ----------------------------------------------------------------------------
# host-side shard prep (slicing / layout / dtype-cast only)
# ----------------------------------------------------------------------------

def host_prep(inputs):
    f32 = np.float32
    adj_0 = np.asarray(inputs["adj_0"], f32)
    noise = np.asarray(inputs["noise"], f32)
    t = np.asarray(inputs["t"]).astype(np.int32)
    w = {k: np.asarray(v, f32) for k, v in inputs.items()
         if k not in ("adj_0", "noise", "t")}

    opw2p = np.concatenate([w["op_W2"], np.zeros((H, 1), f32)], axis=1)
    opb2p = np.concatenate([w["op_b2"], np.zeros((1,), f32)])
    ttab = np.stack([SQRT_AB, SQRT_OMAB], axis=1)          # [100, 2]
    pidx = np.arange(128, dtype=np.int32)[:, None]
    emat = np.zeros((4, 128), f32)
    for kl in range(4):
        emat[kl, 32 * kl:32 * kl + 32] = 1.0
    iden = np.eye(128, dtype=np_bf16)
    jrow = np.arange(N, dtype=f32)[None, :]                # [1, 512]

    shared = {
        "emat": emat, "pidx": pidx, "ttab": ttab,
        "iden": iden, "jrow": jrow,
        "msg_W16": w["msg_W"].astype(np_bf16),
        "upd_W16": w["upd_W"].astype(np_bf16),
        "msgb16": w["msg_b"].reshape(1, L * H).astype(np_bf16),
        "te_W1": w["te_W1"], "te_b1": w["te_b1"],
        "te_W2": w["te_W2"], "te_b2": w["te_b2"],
        "ip_W": w["ip_W"], "ip_b": w["ip_b"],
        "msg_W": w["msg_W"], "msg_b": w["msg_b"],
        "upd_W": w["upd_W"], "upd_b": w["upd_b"],
        "ln_g": w["ln_g"], "ln_b": w["ln_b"],
        "tp_W": w["tp_W"], "tp_b": w["tp_b"],
        "op_W1": w["op_W1"], "op_b1": w["op_b1"],
    }

    in_maps = []
    for c in range(NC):
        sl = slice(c * BS, (c + 1) * BS)
        rows = c + NC * np.arange(NROW)

        eidx_c = EIDX[rows]                                # [64, 512]
        w2pad = opw2p[:, eidx_c]                           # [256, 64, 512]
        w2c = (w2pad.reshape(2, 128, NT, 4, N)
               .transpose(0, 2, 3, 1, 4).astype(np_bf16))  # [2, 16, 4, 128, 512]
        b2p = opb2p[eidx_c].reshape(NT, 4, N).copy()       # [16, 4, 512]

        tgt = noise[:, rows, :].transpose(1, 0, 2).reshape(NT, 4 * B, N).copy()
        rowv = np.zeros((128, NT), f32)
        for kl in range(4):
            rowv[32 * kl:32 * kl + 32, :] = (8 * (4 * np.arange(NT) + kl) + c)[None, :]

        m = dict(shared)
        m.update({
            "adjT0": adj_0[sl].transpose(0, 2, 1).reshape(BS, 4, 128, N).copy(),
            "noiseT": noise[sl].transpose(0, 2, 1).reshape(BS, 4, 128, N).copy(),
            "tgt": tgt, "rowv": rowv,
            "w2c": w2c, "b2p": b2p,
            "tsh": t[sl].copy(),
        })
        in_maps.append(m)
    return in_maps


# ----------------------------------------------------------------------------
# device program (single SPMD program; per-core differences are input data)
# ----------------------------------------------------------------------------

def build_program(debug=False, linearize=False):
    nc = bacc.Bacc("TRN2", target_bir_lowering=False, debug=False, num_devices=NC)
    d = {}

    def din(name, shape, dt=FP):
        d[name] = nc.declare_dram_parameter(name, list(shape), dt, isOutput=False)

    din("adjT0", [BS, 4, 128, N])
    din("noiseT", [BS, 4, 128, N])
    din("tgt", [NT, 128, N])
    din("rowv", [128, NT])
    din("w2c", [2, NT, 4, 128, N], BF)
    din("b2p", [NT, 4, N])
    din("emat", [4, 128])
    din("iden", [128, 128], BF)
    din("jrow", [1, N])
    din("msg_W16", [L, H, H], BF)
    din("upd_W16", [L, 2 * H, H], BF)
    din("msgb16", [1, L * H], BF)
    din("pidx", [128, 1], I32)
    din("ttab", [T, 2])
    din("tsh", [BS], I32)
    din("te_W1", [1, H]); din("te_b1", [H])
    din("te_W2", [H, H]); din("te_b2", [H])
    din("ip_W", [N, H]); din("ip_b", [H])
    din("msg_W", [L, H, H]); din("msg_b", [L, H])
    din("upd_W", [L, 2 * H, H]); din("upd_b", [L, H])
    din("ln_g", [L, H]); din("ln_b", [L, H])
    din("tp_W", [L, H, H]); din("tp_b", [L, H])
    din("op_W1", [H, H]); din("op_b1", [H])

    d["out"] = nc.declare_dram_parameter("out", [1, 1], FP, isOutput=True)
    if debug:
        for nm, shp in DBG_SPECS:
            d[nm] = nc.declare_dram_parameter(nm, list(shp), FP, isOutput=True)

    d["g_loc"] = nc.dram_tensor("g_loc", [H, BS], BF)
    d["g_all"] = nc.dram_tensor("g_all", [NC * H, BS], BF, addr_space="Shared")

    with tile.TileContext(nc, linearize=linearize) as tc:
        _body(nc, tc, d, debug)
    nc.compile()
    return nc


def _body(nc, tc, d, debug):
    f32 = FP
    with ExitStack() as ctx:
        wp = ctx.enter_context(tc.tile_pool(name="wp", bufs=1))
        big = ctx.enter_context(tc.tile_pool(name="big", bufs=2))
        med = ctx.enter_context(tc.tile_pool(name="med", bufs=2))
        sm = ctx.enter_context(tc.tile_pool(name="sm", bufs=2))
        tiny = ctx.enter_context(tc.tile_pool(name="tiny", bufs=2))
        hfp = ctx.enter_context(tc.tile_pool(name="hfp", bufs=4))
        anp = ctx.enter_context(tc.tile_pool(name="anp", bufs=3))
        w2p = ctx.enter_context(tc.tile_pool(name="w2p", bufs=3))
        tgb = ctx.enter_context(tc.tile_pool(name="tgb", bufs=3))
        psA = ctx.enter_context(tc.tile_pool(name="psA", bufs=1, space="PSUM"))
        psB = ctx.enter_context(tc.tile_pool(name="psB", bufs=1, space="PSUM"))
        psC = ctx.enter_context(tc.tile_pool(name="psC", bufs=1, space="PSUM"))
        psS = ctx.enter_context(tc.tile_pool(name="psS", bufs=2, space="PSUM"))

        _ldn = [0]

        def load(pl, shape, dt=FP, src=None, in_ap=None):
            _ldn[0] += 1
            tl = pl.tile(list(shape), dt, tag=f"ld{_ldn[0]}_{src or 'ap'}")
            nc.sync.dma_start(tl[:], in_ap if in_ap is not None else d[src][:])
            return tl

        # ------------- resident weights / constants -------------
        teW1 = load(wp, [1, H], src="te_W1")
        teW2 = load(wp, [128, 2, H], in_ap=d["te_W2"][:].rearrange("(c p) h -> p c h", p=128))
        ipW = load(wp, [128, 4, H], in_ap=d["ip_W"][:].rearrange("(c p) h -> p c h", p=128))
        msgW = load(wp, [128, L, 2, H], BF, in_ap=d["msg_W16"][:].rearrange("l (c p) h -> p l c h", p=128))
        updW = load(wp, [128, L, 4, H], BF, in_ap=d["upd_W16"][:].rearrange("l (c p) h -> p l c h", p=128))
        tpW = load(wp, [128, L, 2, H], in_ap=d["tp_W"][:].rearrange("l (c p) h -> p l c h", p=128))
        opW1 = load(wp, [128, 2, H], in_ap=d["op_W1"][:].rearrange("(c p) h -> p c h", p=128))
        teb1 = load(wp, [128, 2], in_ap=d["te_b1"][:].rearrange("(c p) -> p c", p=128))
        teb2 = load(wp, [128, 2], in_ap=d["te_b2"][:].rearrange("(c p) -> p c", p=128))
        ipb = load(wp, [128, 2], in_ap=d["ip_b"][:].rearrange("(c p) -> p c", p=128))
        updb = load(wp, [128, L, 2], in_ap=d["upd_b"][:].rearrange("l (c p) -> p l c", p=128))
        tpb = load(wp, [128, L, 2], in_ap=d["tp_b"][:].rearrange("l (c p) -> p l c", p=128))
        lnb = load(wp, [128, L, 2], in_ap=d["ln_b"][:].rearrange("l (c p) -> p l c", p=128))
        opb1 = load(wp, [128, 2], in_ap=d["op_b1"][:].rearrange("(c p) -> p c", p=128))
        msgb_r = load(wp, [1, L * H], BF, src="msgb16")
        lng_r = load(wp, [1, L * H], in_ap=d["ln_g"][:].rearrange("l h -> (l h)")[None, :])
        emat = load(wp, [4, 128], src="emat")
        iden = load(wp, [128, 128], BF, src="iden")
        ttab = load(wp, [T, 2], src="ttab")
        pidx = load(wp, [128, 1], I32, src="pidx")
        rowv = load(wp, [128, NT], src="rowv")

        ones_c = wp.tile([128, 1], f32)
        nc.vector.memset(ones_c[:], 1.0)
        ones_r = wp.tile([1, 128], f32)
        nc.vector.memset(ones_r[:], 1.0)
        eps1 = wp.tile([1, 1], f32)
        nc.vector.memset(eps1[:], 1e-5)
        ones_c16 = wp.tile([128, 1], BF)
        nc.vector.memset(ones_c16[:], 1.0)

        # j-index broadcast map [128, N] (for the on-device triangle mask)
        jb_ps = psS.tile([128, N], f32, tag="ps_small")
        jrow_sb = sm.tile([1, N], f32, tag="jrow")
        nc.sync.dma_start(jrow_sb[:], d["jrow"][:])
        nc.tensor.matmul(jb_ps[:], ones_r[:], jrow_sb[:], start=True, stop=True)
        jb = wp.tile([128, N], f32)
        nc.scalar.copy(jb[:], jb_ps[:])

        # ------------- t path -------------
        tb = tiny.tile([T, BS], I32, tag="tb")
        nc.sync.dma_start(tb[:], d["tsh"][None, :].to_broadcast([T, BS]))
        oneh = tiny.tile([T, BS], f32, tag="oneh")
        nc.vector.tensor_tensor(out=oneh[:], in0=pidx[:][0:T, :].to_broadcast([T, BS]),
                                in1=tb[:], op=ALU.is_equal)
        sasb_ps = psS.tile([1, 2 * BS], f32, tag="ps_small")
        nc.tensor.matmul(sasb_ps[:, 0:BS], ttab[:, 0:1], oneh[:], start=True, stop=True)
        nc.tensor.matmul(sasb_ps[:, BS:2 * BS], ttab[:, 1:2], oneh[:], start=True, stop=True)
        sasb = tiny.tile([1, 2 * BS], f32, tag="sasb")
        nc.scalar.copy(sasb[:], sasb_ps[:])
        if debug:
            nc.sync.dma_start(d["dbg_sasb"][:].rearrange("a s -> (a s)")[None, :], sasb[:])
        sab_ps = psS.tile([128, 2 * BS], f32, tag="ps_small")
        nc.tensor.matmul(sab_ps[:], ones_r[:], sasb[:], start=True, stop=True)
        saB = tiny.tile([128, 2 * BS], f32, tag="saB")
        nc.scalar.copy(saB[:], sab_ps[:])

        tff = tiny.tile([1, BS], f32, tag="tf")
        nc.vector.tensor_copy(tff[:], tb[0:1, :])
        nc.vector.tensor_scalar_mul(out=tff[:], in0=tff[:], scalar1=1.0 / T)

        x1_ps = psS.tile([128, 2 * BS], f32, tag="ps_small")
        a1 = tiny.tile([128, 2, BS], f32, tag="a1")
        for ch in range(2):
            nc.tensor.matmul(x1_ps[:, ch * BS:(ch + 1) * BS],
                             teW1[:, 128 * ch:128 * (ch + 1)], tff[:], start=True, stop=True)
            nc.scalar.activation(a1[:, ch, :], x1_ps[:, ch * BS:(ch + 1) * BS],
                                 AF.Silu, bias=teb1[:, ch:ch + 1])
        te_ps = psS.tile([128, 2 * BS], f32, tag="ps_small")
        for mt in range(2):
            for kc in range(2):
                nc.tensor.matmul(te_ps[:, mt * BS:(mt + 1) * BS],
                                 teW2[:, kc, 128 * mt:128 * (mt + 1)], a1[:, kc, :],
                                 start=(kc == 0), stop=(kc == 1))
        teF = tiny.tile([128, 2, BS], f32, tag="teF")
        for mt in range(2):
            nc.scalar.activation(teF[:, mt, :], te_ps[:, mt * BS:(mt + 1) * BS],
                                 AF.Identity, bias=teb2[:, mt:mt + 1])

        tbF = wp.tile([128, L, 2, BS], f32)
        for l in range(L):
            cb = tiny.tile([128, 2], f32, tag="cb")
            nc.vector.tensor_add(cb[:], tpb[:, l, :], updb[:, l, :])
            tp_ps = psS.tile([128, 2 * BS], f32, tag="ps_small")
            for mt in range(2):
                for kc in range(2):
                    nc.tensor.matmul(tp_ps[:, mt * BS:(mt + 1) * BS],
                                     tpW[:, l, kc, 128 * mt:128 * (mt + 1)], teF[:, kc, :],
                                     start=(kc == 0), stop=(kc == 1))
                nc.scalar.activation(tbF[:, l, mt, :], tp_ps[:, mt * BS:(mt + 1) * BS],
                                     AF.Identity, bias=cb[:, mt:mt + 1])

        ars4 = wp.tile([1, BS, N], BF)
        hg = wp.tile([128, 2, BS], f32)
        inv_h = 1.0 / H

        def prep_sample(s):
            a0 = big.tile([128, 4, N], f32, tag="a0")
            nT = big.tile([128, 4, N], f32, tag="nT")
            nc.sync.dma_start(a0[:], d["adjT0"][s].rearrange("c p n -> p c n"))
            nc.sync.dma_start(nT[:], d["noiseT"][s].rearrange("c p n -> p c n"))
            nc.gpsimd.tensor_scalar_mul(out=nT[:], in0=nT[:],
                                        scalar1=saB[:, BS + s:BS + s + 1])
            # adjT computed in place over a0
            nc.vector.scalar_tensor_tensor(out=a0[:], in0=a0[:],
                                           scalar=saB[:, s:s + 1], in1=nT[:],
                                           op0=ALU.mult, op1=ALU.add)
            adjT = a0
            absT = big.tile([128, 4, N], f32, tag="absT")
            nc.scalar.activation(absT[:], adjT[:], AF.Abs)
            rs_ps = psS.tile([1, N], f32, tag="ps_small")
            for jc in range(4):
                nc.tensor.matmul(rs_ps[:], ones_c[:], absT[:, jc, :],
                                 start=(jc == 0), stop=(jc == 3))
            vec = sm.tile([1, 3, N], f32, tag="vec")
            nc.scalar.activation(vec[:, 0, :], rs_ps[:], AF.Identity, bias=1.0)
            nc.vector.reciprocal_approx_accurate(out=vec[:, 1, :], in_=vec[:, 0, :],
                                                 scratch=vec[:, 2, :])
            if debug and s == 0:
                nc.sync.dma_start(d["dbg_rinv"][:], vec[:, 1, :])
            rb_ps = psS.tile([128, N], f32, tag="ps_small")
            nc.tensor.matmul(rb_ps[:], ones_r[:], vec[:, 1, :], start=True, stop=True)
            AnT = anp.tile([128, 4, N], BF, tag="AnT")
            nc.vector.tensor_tensor(out=AnT[:], in0=adjT[:],
                                    in1=rb_ps[:][:, None, :].to_broadcast([128, 4, N]),
                                    op=ALU.mult)
            ars_ps = psS.tile([1, N], f32, tag="ps_small")
            for jc in range(4):
                nc.tensor.matmul(ars_ps[:], ones_c16[:], AnT[:, jc, :],
                                 start=(jc == 0), stop=(jc == 3))
            nc.scalar.copy(ars4[:, s, :], ars_ps[:])
            h_ps = psC.tile([128, 2, N], f32, tag="hnew")
            for ft in range(2):
                for jc in range(4):
                    nc.tensor.matmul(h_ps[:, ft, :], ipW[:, jc, 128 * ft:128 * (ft + 1)],
                                     adjT[:, jc, :], start=(jc == 0), stop=(jc == 3))
            hF = hfp.tile([128, 2, N], BF, tag="hF")
            for ft in range(2):
                nc.scalar.activation(hF[:, ft, :], h_ps[:, ft, :], AF.Identity,
                                     bias=ipb[:, ft:ft + 1])
            if debug and s == 0:
                nc.gpsimd.dma_start(d["dbg_h0"][:], hF[:])
            return AnT, hF

        def layer(l, s, AnT, hF):
            mp_ps = psA.tile([128, 4, H], f32, tag="mp")
            for jt in range(4):
                for kc in range(2):
                    nc.tensor.matmul(mp_ps[:, jt, :],
                                     hF[:, kc, 128 * jt:128 * (jt + 1)],
                                     msgW[:, l, kc, :], start=(kc == 0), stop=(kc == 1))
            mpS = med.tile([128, 4, H], BF, tag="mpS")
            nc.vector.tensor_copy(mpS[:], mp_ps[:])
            msg_ps = psB.tile([128, 2, N], f32, tag="msg")
            for ft in range(2):
                for jc in range(4):
                    nc.tensor.matmul(msg_ps[:, ft, :],
                                     mpS[:, jc, 128 * ft:128 * (ft + 1)],
                                     AnT[:, jc, :], start=(jc == 0), stop=False)
                nc.tensor.matmul(msg_ps[:, ft, :],
                                 msgb_r[:, l * H + 128 * ft:l * H + 128 * (ft + 1)],
                                 ars4[:, s, :], start=False, stop=True)
            msgS = med.tile([128, 2, N], BF, tag="msgS")
            nc.vector.tensor_copy(msgS[:], msg_ps[:])
            hn_ps = psC.tile([128, 2, N], f32, tag="hnew")
            for ft in range(2):
                for kc in range(4):
                    rhs = hF[:, kc, :] if kc < 2 else msgS[:, kc - 2, :]
                    nc.tensor.matmul(hn_ps[:, ft, :],
                                     updW[:, l, kc, 128 * ft:128 * (ft + 1)],
                                     rhs, start=(kc == 0), stop=False)
                nc.tensor.matmul(hn_ps[:, ft, :], iden[:],
                                 hF[:, ft, :], start=False, stop=True)
            xS = med.tile([128, 2, N], f32, tag="xS")
            for ft in range(2):
                nc.scalar.activation(xS[:, ft, :], hn_ps[:, ft, :], AF.Identity,
                                     bias=tbF[:, l, ft, s:s + 1])
            xq = med.tile([128, 2, N], f32, tag="xq")
            nc.scalar.activation(xq[:], xS[:], AF.Square)
            s1_ps = psS.tile([1, N], f32, tag="ps_small")
            for ft in range(2):
                nc.tensor.matmul(s1_ps[:], ones_c[:], xS[:, ft, :],
                                 start=(ft == 0), stop=(ft == 1))
            s2_ps = psS.tile([1, N], f32, tag="ps_small")
            for ft in range(2):
                nc.tensor.matmul(s2_ps[:], ones_c[:], xq[:, ft, :],
                                 start=(ft == 0), stop=(ft == 1))
            vec = sm.tile([1, 3, N], f32, tag="vec")
            # vec0 = -mu ; vec1 = mu^2 -> var -> sd ; vec2 = rstd ; vec0 *= rstd
            nc.vector.tensor_scalar_mul(out=vec[:, 0, :], in0=s1_ps[:], scalar1=-inv_h)
            nc.scalar.activation(vec[:, 1, :], vec[:, 0, :], AF.Square)
            nc.vector.scalar_tensor_tensor(out=vec[:, 1, :], in0=s2_ps[:], scalar=inv_h,
                                           in1=vec[:, 1, :], op0=ALU.mult, op1=ALU.subtract)
            nc.scalar.activation(vec[:, 1, :], vec[:, 1, :], AF.Sqrt, bias=eps1[:])
            nc.vector.reciprocal_approx_fast(out=vec[:, 2, :], in_=vec[:, 1, :])
            nc.vector.tensor_mul(vec[:, 0, :], vec[:, 0, :], vec[:, 2, :])
            hF_new = hfp.tile([128, 2, N], BF, tag="hF")
            for ft in range(2):
                G_ps = psS.tile([128, N], f32, tag="ps_small")
                nc.tensor.matmul(G_ps[:], lng_r[:, l * H + 128 * ft:l * H + 128 * (ft + 1)],
                                 vec[:, 2, :], start=True, stop=True)
                D_ps = psS.tile([128, N], f32, tag="ps_small")
                nc.tensor.matmul(D_ps[:], lng_r[:, l * H + 128 * ft:l * H + 128 * (ft + 1)],
                                 vec[:, 0, :], start=True, stop=True)
                Dt = sm.tile([128, N], f32, tag="Dt")
                nc.scalar.activation(Dt[:], D_ps[:], AF.Identity,
                                     bias=lnb[:, l, ft:ft + 1])
                y1 = sm.tile([128, N], f32, tag="y1")
                nc.vector.tensor_mul(y1[:], xS[:, ft, :], G_ps[:])
                nc.gpsimd.tensor_add(hF_new[:, ft, :], y1[:], Dt[:])
            return hF_new

        for pair in range(BS // 2):
            ss = [2 * pair, 2 * pair + 1]
            states = {}
            for s in ss:
                states[s] = prep_sample(s)
            for l in range(L):
                for s in ss:
                    AnT, hF = states[s]
                    hF_new = layer(l, s, AnT, hF)
                    states[s] = (AnT, hF_new)
            for s in ss:
                _, hF = states[s]
                if debug and s == 0:
                    nc.gpsimd.dma_start(d["dbg_hf"][:], hF[:])
                hgt = tiny.tile([128, 2], f32, tag="hgt")
                nc.vector.reduce_sum(hgt[:], hF[:], axis=mybir.AxisListType.X)
                nc.vector.tensor_copy(hg[:, :, s:s + 1], hgt[:][:, :, None])

        # ------------- pooling head -------------
        go_ps = psS.tile([128, 2 * BS], f32, tag="ps_small")
        for mt in range(2):
            for kc in range(2):
                nc.tensor.matmul(go_ps[:, mt * BS:(mt + 1) * BS],
                                 opW1[:, kc, 128 * mt:128 * (mt + 1)], hg[:, kc, :],
                                 start=(kc == 0), stop=(kc == 1))
        gof = tiny.tile([128, 2, BS], f32, tag="gof")
        for mt in range(2):
            nc.scalar.activation(gof[:, mt, :], go_ps[:, mt * BS:(mt + 1) * BS],
                                 AF.Silu, bias=opb1[:, mt:mt + 1])
        if debug:
            nc.sync.dma_start(d["dbg_gof"][:], gof[:])
        gof16 = tiny.tile([128, 2, BS], BF, tag="gof16")
        nc.vector.tensor_copy(gof16[:], gof[:])
        nc.sync.dma_start(d["g_loc"][:].rearrange("(c p) s -> p c s", p=128), gof16[:])
        nc.gpsimd.collective_compute(
            "AllGather", ALU.bypass,
            ins=[d["g_loc"][:]], outs=[d["g_all"][:]],
            replica_groups=[list(range(NC))])
        gA = wp.tile([128, 2, B], BF)
        g_all_v = d["g_all"][:].rearrange("(r c p) s -> c p r s", r=NC, p=128)
        for ch in range(2):
            nc.sync.dma_start(gA[:, ch, :].rearrange("p (r s) -> p r s", r=NC),
                              g_all_v[ch])

        # ------------- phase B: pred, masked diff, SSE -------------
        sseb = wp.tile([128, NT], f32)
        dsq = wp.tile([128, N], f32)
        for ti in range(NT):
            w2t0 = w2p.tile([128, 4, N], BF, tag="w2a")
            w2t1 = w2p.tile([128, 4, N], BF, tag="w2b")
            nc.sync.dma_start(w2t0[:], d["w2c"][0, ti].rearrange("kl p n -> p kl n"))
            nc.sync.dma_start(w2t1[:], d["w2c"][1, ti].rearrange("kl p n -> p kl n"))
            b2t = tgb.tile([4, N], f32, tag="b2t")
            nc.sync.dma_start(b2t[:], d["b2p"][ti])
            tgt_t = tgb.tile([128, N], f32, tag="tgt")
            nc.sync.dma_start(tgt_t[:], d["tgt"][ti])
            tgm = tgb.tile([128, N], f32, tag="tgm")
            nc.vector.scalar_tensor_tensor(out=tgm[:], in0=jb[:],
                                           scalar=rowv[:, ti:ti + 1], in1=tgt_t[:],
                                           op0=ALU.is_gt, op1=ALU.mult)
            pp = psS.tile([128, N], f32, tag="ps_small")
            for kl in range(4):
                nc.tensor.matmul(pp[32 * kl:32 * kl + 32, :], gA[:, 0, :], w2t0[:, kl, :],
                                 start=True, stop=False, tile_position=(0, 32 * kl))
                nc.tensor.matmul(pp[32 * kl:32 * kl + 32, :], gA[:, 1, :], w2t1[:, kl, :],
                                 start=False, stop=False, tile_position=(0, 32 * kl))
            nc.tensor.matmul(pp[:], emat[:], b2t[:], start=False, stop=True,
                             skip_group_check=True)
            dt_ = sm.tile([128, N], f32, tag="d")
            nc.vector.tensor_sub(dt_[:], pp[:], tgm[:])
            if debug and ti == 0:
                nc.sync.dma_start(d["dbg_d0"][:], dt_[:])
            nc.scalar.activation(dsq[:], dt_[:], AF.Square, accum_out=sseb[:, ti:ti + 1])
        if debug:
            nc.sync.dma_start(d["dbg_sse"][:], sseb[:])

        ssev = tiny.tile([128, 1], f32, tag="ssev")
        nc.vector.reduce_sum(ssev[:], sseb[:], axis=mybir.AxisListType.X)
        tot_ps = psS.tile([1, 1], f32, tag="ps_small")
        nc.tensor.matmul(tot_ps[:], ssev[:], ones_c[:], start=True, stop=True)
        outv = tiny.tile([1, 1], f32, tag="outv")
        nc.scalar.activation(outv[:], tot_ps[:], AF.Identity, scale=1.0 / (B * NE))
        nc.sync.dma_start(d["out"][:], outv[:])


# ----------------------------------------------------------------------------
# entry point
# ----------------------------------------------------------------------------

_NC_CACHE = {}


def get_program(debug=False, linearize=False):
    key = (debug, linearize)
    if key not in _NC_CACHE:
        _NC_CACHE[key] = build_program(debug, linearize)
    return _NC_CACHE[key]


def run(inputs, debug=False, linearize=False, **kw):
    in_maps = host_prep(inputs)
    nc = get_program(debug, linearize)
    res = run_bass_kernel_spmd(nc, in_maps, list(range(NC)), **kw)
    total = np.float64(0.0)
    for c in range(NC):
        total += np.float64(res.results[c]["out"][0, 0])
    return np.float32(total), res


def kernel(**inputs):
    loss, _ = run(inputs)
    return np.asarray(loss, dtype=np.float32)
